# revision 7
# baseline (speedup 1.0000x reference)
"""AdaptiveSudokuLoss on 8 TRN2 NeuronCores — pure data-parallel.

Full inputs: outputs (65536, 81, 9) f32, targets (65536, 81) int64.
Output: scalar f32 loss.

Host preprocessing (layout transforms only, no reductions):
  x -> bf16, pad digit axis 9 -> 10 with -100 (exp -> 0; keeps every run
  even-length/4B-aligned so bf16 tensor ops hit the DVE 2x packed mode).
  xt = x[target] gather (bf16) -- replaces the onehot tensor entirely;
  the device sums it with an idle-PE ones-matmul (half the DMA bytes of
  the onehot approach and no q-add DVE pass).

Math per cell (9 logits x_d):
  e = exp(x); Z = sum_d e; logZ = ln Z; r = exp(-logZ); p = e * r
  loss = (1.1*S_logZ - S_xt - 0.1*S_px)/N + 0.5*S_sq/(B*9*27)
  where S_sq = sum over rows+cols+boxes of (groupsum(p) - 1)^2.

Engine split per tile (vs the 173us baseline):
  ScalarE: exp (per half); ONE per-tile Ln+accum over both halves' Z; the
           r2 pair-duplicate now comes straight out of a single
           Exp(-lnZ) pass reading a broadcast AP (the old kernel paid a
           separate exp + Copy); TWO merged Square+accum per tile
           (rows+cols fused, box) instead of three.
  DVE:     Z chain as 3 packed 2x adds (4th add on GpSimd); p = e*r2
           (2x); fused x*p AND its full sum in ONE bf16
           scalar_tensor_tensor pass (accum_out) -- bf16 because the STT
           accumulator path runs 1x with non-bf16 sources.
  GpSimd:  final Z add (st), otherwise idle.
  PE:      27 constraint group sums (identity matmuls into PSUM) and the
           xt sum (ones matmul). The old x*q ones-matmul stream is gone
           (folded into the DVE STT pass).

Each core processes 8192 samples, emits partial sums as [128, 8] f32;
host combines. No collectives.
"""
import numpy as np
import ml_dtypes

import concourse.bass as bass
import concourse.bass_utils as bass_utils_mod
import concourse.tile as tile_mod
from concourse import mybir
from concourse.bass_utils import run_bass_kernel_spmd
from concourse.masks import make_identity
from concourse.vector_clock import ScopedClock

BF16NP = ml_dtypes.bfloat16

# ---------------------------------------------------------------- tile fix --
# walrus (b16 2026-05-04) accepts only one sem-wait per instruction; Tile's
# add_semaphores attaches several. Hoist extras onto same-engine NOPs.

_nop_counter = [0]


def _split_multi_waits(nc):
    for fn in nc.m.functions:
        for bb in fn.blocks:
            out = []
            changed = False
            for inst in bb.instructions:
                si = inst.sync_info
                if si is not None and len(si.on_wait) > 1:
                    waits = list(si.on_wait)
                    for w in waits[:-1]:
                        _nop_counter[0] += 1
                        n = mybir.InstNoOp(
                            name=f"I-waitsplit-{_nop_counter[0]}", ins=[], outs=[])
                        n.engine = inst.engine
                        n.sync_info = mybir.SyncInfo(on_wait=[w], on_update=[])
                        out.append(n)
                    si.on_wait = waits[-1:]
                    inst.sync_info = si
                    changed = True
                out.append(inst)
            if changed:
                bb.instructions = out


def _patched_drain_and_barrier(self, tick_clock, wait_clock):
    nc = self.nc
    probe = nc.sync.nop()
    wait_clock.add_sem_waits(probe.ins, ScopedClock({None: tick_clock.global_clock}))
    nc.sync.drain()
    nc.all_engine_barrier()
    assert self.sems is not None
    popped = nc._tile_sem_poison_stack.pop()
    assert popped is self._sem_poison
    nc.clear_and_free_semaphores(list(self.sems.allocated().values()))
    nc.all_engine_barrier()
    _split_multi_waits(nc)


tile_mod.TileContext._drain_and_barrier = _patched_drain_and_barrier

# ------------------------------------------------------------------- consts --
B = 65536
NCORES = 8
BS = B // NCORES            # samples per core = 8192
P = 128                     # partitions
SPP = BS // P               # samples per partition = 64
CPP = SPP * 81              # cells per partition = 5184
D = 10                      # padded digit axis
FPP = CPP * D               # bf16 elems per partition = 51840
NT = 8                      # tiles
TS = SPP // NT              # samples per partition per tile = 8
TC = TS * 81                # cells per tile = 648
TF = TC * D                 # elems per tile = 6480
HC = TC // 2                # cells per half = 324
HF = HC * D                 # elems per half = 3240

F32 = mybir.dt.float32
F16 = mybir.dt.float16
BF16 = mybir.dt.bfloat16
ALU = mybir.AluOpType
ACTF = mybir.ActivationFunctionType
AX = mybir.AxisListType

USE_POOL_ST = True          # final Z add on GpSimd instead of DVE

_CACHE = {}


def _build():
    nc = bass.Bass()
    cm1 = nc.alloc_sbuf_tensor("const-float32-neg1", [128, 1], F32)
    nc.gpsimd.memset(cm1.ap(), -1.0)
    nc.const_aps.aps[(F32, -1.0)] = cm1.ap()
    ident = nc.alloc_sbuf_tensor("identity-bf16", [128, 128], BF16)
    make_identity(nc, ident.ap())
    ones = nc.alloc_sbuf_tensor("ones-bf16", [128, 1], BF16)
    nc.gpsimd.memset(ones.ap(), 1.0)
    nc.all_engine_barrier()
    x_ext = nc.declare_dram_parameter("x", [P, FPP], BF16, isOutput=False)
    t_ext = nc.declare_dram_parameter("t", [P, CPP], BF16, isOutput=False)
    out_ext = nc.declare_dram_parameter("out", [P, 8], F32, isOutput=True)
    idap = ident.ap()
    onesap = ones.ap()

    with tile_mod.TileContext(nc) as tc:
        with (
            tc.tile_pool(name="inp", bufs=4) as xp_pool,
            tc.tile_pool(name="work", bufs=3) as wp,
            tc.tile_pool(name="deep", bufs=2) as dp,
            tc.tile_pool(name="pers", bufs=1) as pp,
            tc.tile_pool(name="psum", bufs=1, space="PSUM") as qp,
        ):
            accL = pp.tile([P, NT], F32)        # sum logZ   (per tile)
            accSQ = pp.tile([P, 2 * NT], F32)   # sum (g-1)^2 (2 per tile)
            xt_ps = qp.tile([1, 512], F32)      # PE-accumulated sum of xt
            xq_ps = qp.tile([1, 512], F32)      # PE-accumulated sum of x*p

            sq_prev = [None]

            def emit_squares():
                # Two merged Square+accum over a finished tile's PSUM group
                # sums: rows+cols share one pass, box the other. Runs one
                # tile late so the PSUM WAR clears before the next tile's
                # matmuls need the banks.
                kk, gs = sq_prev[0]
                sq1 = wp.tile([P, 4 * HC], BF16)
                s1v = sq1[:].rearrange("p (a s g) -> p a s g", a=4, s=4)
                rc = gs[:, 0:4, :].rearrange("p a (s q) -> p a s q", s=4)
                nc.scalar.activation(s1v, rc[:, :, :, 0:81], ACTF.Square,
                                     bias=-1.0,
                                     accum_out=accSQ[:, 2 * kk:2 * kk + 1])
                sq2 = wp.tile([P, 648], BF16)
                s2v = sq2[:].rearrange("p (a g) -> p a g", a=24)
                bx = gs[:, 4:6, :].rearrange("p a q -> p (a q)")[:, 0:768] \
                    .rearrange("p (a q) -> p a q", q=32)
                nc.scalar.activation(s2v, bx[:, :, 0:27], ACTF.Square,
                                     bias=-1.0,
                                     accum_out=accSQ[:, 2 * kk + 1:
                                                     2 * kk + 2])

            def emit_front(k):
                xin = xp_pool.tile([P, TF], BF16)
                nc.sync.dma_start(xin[:], x_ext[:, k * TF:(k + 1) * TF])
                xtin = xp_pool.tile([P, TC], BF16)
                nc.sync.dma_start(xtin[:], t_ext[:, k * TC:(k + 1) * TC])
                sts = wp.tile([P, TC], F32)     # Z, both halves
                ets = []
                for h in (0, 1):
                    xh = xin[:, h * HF:(h + 1) * HF]
                    et = wp.tile([P, HF], BF16)
                    ets.append(et)
                    nc.scalar.activation(et[:], xh, ACTF.Exp)
                    e3 = et[:].rearrange("p (c d) -> p c d", d=D)
                    # Z per cell via packed 2x adds:
                    # t1[j] = e[j]+e[j+4] (j=0..3); t1[0:2] += e[8:10];
                    # t1[0:2] += t1[2:4]; Z = t1[0]+t1[1]  (last on Pool)
                    t1 = wp.tile([P, HC * 4], BF16)
                    t1v = t1[:].rearrange("p (c f) -> p c f", f=4)
                    nc.vector.tensor_tensor(t1v, e3[:, :, 0:4], e3[:, :, 4:8],
                                            op=ALU.add)
                    nc.vector.tensor_tensor(t1v[:, :, 0:2], t1v[:, :, 0:2],
                                            e3[:, :, 8:10], op=ALU.add)
                    nc.vector.tensor_tensor(t1v[:, :, 0:2], t1v[:, :, 0:2],
                                            t1v[:, :, 2:4], op=ALU.add)
                    stv = sts[:, h * HC:(h + 1) * HC].unsqueeze(2)
                    if USE_POOL_ST:
                        nc.gpsimd.tensor_tensor(stv, t1v[:, :, 0:1],
                                                t1v[:, :, 1:2], op=ALU.add)
                    else:
                        nc.vector.tensor_tensor(stv, t1v[:, :, 0:1],
                                                t1v[:, :, 1:2], op=ALU.add)
                return (k, xin, xtin, sts, ets)

            def emit_back(carry):
                k, xin, xtin, sts, ets = carry
                # one per-tile ln over both halves' Z; r2 = exp(-lnZ)
                # pair-duplicated straight from a broadcast-AP read.
                lst = wp.tile([P, TC], F32)
                nc.scalar.activation(lst[:], sts[:], ACTF.Ln,
                                     accum_out=accL[:, k:k + 1])
                r2 = wp.tile([P, TC * 2], BF16)
                r2v = r2[:].rearrange("p (c t) -> p c t", t=2)
                nc.scalar.activation(
                    r2v, lst[:].unsqueeze(2).broadcast_to([P, TC, 2]),
                    ACTF.Exp, scale=-1.0)

                pt = dp.tile([P, TF], BF16)
                # PSUM: slots 0,1 rows (h0,h1); 2,3 cols; 4,5 box region
                gs = qp.tile([P, 6, 512], F32)
                p6 = pt[:].rearrange("p (s r c d) -> p s r c d",
                                     s=TS, r=9, c=9, d=D)
                for h in (0, 1):
                    xh = xin[:, h * HF:(h + 1) * HF]
                    et = ets[h]
                    p5 = pt[:, h * HF:(h + 1) * HF].rearrange(
                        "p (c j t) -> p c j t", j=D // 2, t=2)
                    e5 = et[:].rearrange("p (c j t) -> p c j t",
                                         j=D // 2, t=2)
                    r5 = r2v[:, h * HC:(h + 1) * HC, :].unsqueeze(2) \
                        .broadcast_to([P, HC, D // 2, 2])
                    nc.vector.tensor_tensor(p5, e5, r5, op=ALU.mult)

                    # x*p at 2x; dst reuses the dead e tile; summed by the
                    # PE below (the fused STT runs 1x -- no 2x uop)
                    nc.vector.tensor_tensor(et[:], xh,
                                            pt[:, h * HF:(h + 1) * HF],
                                            op=ALU.mult)

                    # rows/cols group sums for this half's 4 samples
                    ssl = slice(4 * h, 4 * h + 4)
                    gr = gs[:, 0 + h, :].rearrange("p (s q) -> p s q", s=4)
                    gc = gs[:, 2 + h, :].rearrange("p (s q) -> p s q", s=4)
                    for c in range(9):
                        nc.tensor.matmul(
                            gr[:, :, 0:81], idap, p6[:, ssl, :, c, 0:9],
                            start=(c == 0), stop=(c == 8))
                        nc.tensor.matmul(
                            gc[:, :, 0:81], idap, p6[:, ssl, c, :, 0:9],
                            start=(c == 0), stop=(c == 8))
                    # x*p sum chunks into the xq PSUM bank
                    for c2 in range(7):
                        n = 512 if c2 < 6 else HF - 6 * 512
                        nc.tensor.matmul(
                            xq_ps[0:1, 0:n], onesap,
                            et[:, 512 * c2:512 * c2 + n],
                            start=(k == 0 and h == 0 and c2 == 0),
                            stop=(k == NT - 1 and h == 1 and c2 == 6))

                # boxes at tile granularity: R regions are 1KB each
                p8 = pt[:].rearrange("p (s R i C j d) -> p s R i C j d",
                                     s=TS, R=3, i=3, C=3, j=3, d=D)
                gb = gs[:, 4:6, :].rearrange("p a q -> p (a q)")[:, 0:768] \
                    .rearrange("p (R s q) -> p R s q", R=3, s=TS)
                for R in range(3):
                    for ij in range(9):
                        i, j = divmod(ij, 3)
                        nc.tensor.matmul(
                            gb[:, R, :, 0:27], idap,
                            p8[:, :, R, i, :, j, 0:9],
                            start=(ij == 0), stop=(ij == 8))

                # xt sum for this tile into its own PSUM bank
                for c2 in (0, 1):
                    n = 512 if c2 == 0 else TC - 512
                    nc.tensor.matmul(
                        xt_ps[0:1, 0:n], onesap,
                        xtin[:, 512 * c2:512 * c2 + n],
                        start=(k == 0 and c2 == 0),
                        stop=(k == NT - 1 and c2 == 1))

                if sq_prev[0] is not None:
                    emit_squares()
                sq_prev[0] = (k, gs)

            # software pipeline: front-end of tile k overlaps the ScalarE
            # ln/r2 + back-end of tile k-1
            carry = None
            for k in range(NT):
                front = emit_front(k)
                if carry is not None:
                    emit_back(carry)
                carry = front
            emit_back(carry)
            emit_squares()

            ot = pp.tile([P, 8], F32)
            nc.vector.memset(ot[:], 0.0)
            nc.vector.tensor_reduce(ot[:, 0:1], accL[:], axis=AX.X, op=ALU.add)
            nc.vector.tensor_reduce(ot[0:1, 1:2], xq_ps[:], axis=AX.X,
                                    op=ALU.add)
            nc.vector.tensor_reduce(ot[0:1, 2:3], xt_ps[:], axis=AX.X,
                                    op=ALU.add)
            nc.vector.tensor_reduce(ot[:, 3:4], accSQ[:], axis=AX.X,
                                    op=ALU.add)
            nc.sync.dma_start(out_ext[:], ot[:])
    return nc


def _get_nc():
    if "nc" not in _CACHE:
        _CACHE["nc"] = _build()
    return _CACHE["nc"]


def _prep_x(outputs):
    """(B, 81, 9) f32 -> per-core [128, FPP] bf16 with digit pad -100."""
    xb = np.full((B, 81, D), -100.0, dtype=BF16NP)
    xb[:, :, :9] = outputs.astype(BF16NP)
    return xb.reshape(NCORES, P, FPP)


def _prep_xt(outputs, targets):
    """x[target] gather -> per-core [128, CPP] bf16 (order irrelevant)."""
    xt = np.take_along_axis(outputs, targets[..., None].astype(np.int64),
                            axis=2)[..., 0]
    return np.ascontiguousarray(xt.astype(BF16NP).reshape(NCORES, P, CPP))


def kernel(outputs: np.ndarray, targets: np.ndarray, _want_results=False,
           **run_kwargs) -> np.ndarray:
    nc = _get_nc()
    outputs = np.ascontiguousarray(outputs, dtype=np.float32)
    xs_all = _prep_x(outputs)
    ts_all = _prep_xt(outputs, np.ascontiguousarray(targets))
    in_maps = [{"x": xs_all[i], "t": ts_all[i]} for i in range(NCORES)]
    res = run_bass_kernel_spmd(nc, in_maps, core_ids=list(range(NCORES)),
                               **run_kwargs)

    S = np.zeros(8, dtype=np.float64)
    for i in range(NCORES):
        S += res.results[i]["out"].astype(np.float64).sum(axis=0)
    S_logZ, S_px, S_xt, S_sq = S[0], S[1], S[2], S[3]
    N = float(B * 81)
    loss = (1.1 * S_logZ - S_xt - 0.1 * S_px) / N \
        + 0.5 * S_sq / (B * 9.0 * 27.0)
    out = np.asarray(loss, dtype=np.float32)
    if _want_results:
        return out, res
    return out
